# revision 37
# baseline (speedup 1.0000x reference)
"""CrossScaleGNN Trainium2 kernel (8 NeuronCores, SPMD).

Strategy:
  - Nodes partitioned across 8 cores (12544/core incl. padding), 98 tiles of
    128 nodes per core.
  - Edges bucketed by destination tile; sources bucketed into 4 index ranges
    of 25088 rows (int16 dma_gather limit). Gather counts are rounded to 16
    (the index-wrap granularity) per (tile,bucket), max over cores for SPMD;
    slot regions stay 128-aligned for the PE blocks (unwritten tail slots are
    zeroed once at start and masked by S=0). Self-loops are not gathered at
    all: each tile's own y rows come from one contiguous DMA out of the
    core-local y_shard, summed via an identity S block.
  - Per layer: z = x @ W (PE, fp32), y = dinv*z (ACT, ->bf16), AllGather y,
    then per destination tile: dma_gather y[src] rows (4 SWDGE queues),
    build a 0/1 selection matrix S from dst_local via one DVE is_equal with
    step-0 broadcast APs, and segment-sum via PE matmuls accumulating in
    PSUM. Bias enters via a diag(sqrt(deg)) pseudo-block, relu+dinv on ACT.
  - Head: community gather (dma_gather from comm table), sigmoid gate,
    classifier matmul, log_softmax.
Measured (repeat-loop slope, 8 cores): compute phases ~1.75 ms + ~0.08 ms
AllGathers; max rel err vs fp32 JAX reference 1.1e-4.
"""
import sys
import time

sys.path.insert(0, '/opt/trn_rl_repo')

import numpy as np
import ml_dtypes

import concourse.bass as bass
import concourse.bacc as bacc
import concourse.tile as tile
import concourse.mybir as mybir
from concourse.bass_utils import run_bass_kernel_spmd

bf16 = mybir.dt.bfloat16
f32 = mybir.dt.float32
i32 = mybir.dt.int32
i16 = mybir.dt.int16
AF = mybir.ActivationFunctionType
OP = mybir.AluOpType

N = 100000
E = 1600000
NFEAT = 256
NHID = 128
NCLASS = 64
NCOMM = 1000
NC = 8
P = 128
TPC = 98                 # tiles per core
NPC = TPC * P            # nodes per core (12544)
NPAD = NC * NPC          # 100352
NBUCK = 4
BUCK = NPAD // NBUCK     # 25088 rows per src bucket (int16-addressable)

_cache = {}

# phase-A precision: bf16 x/W0 (z0 is rounded to bf16 for the y-table
# anyway); flip to False to restore the all-fp32 phase A.
XA16 = True


def _roundup(x, m):
    return (x + m - 1) // m * m


def _host_prep(node_features, node_adj, node_to_comm_map):
    # self-loops are NOT added as edges: each tile's own y rows are loaded
    # with one contiguous DMA from y_shard and summed via an identity S block.
    src_e = np.asarray(node_adj[0]).astype(np.int64)
    dst_e = np.asarray(node_adj[1]).astype(np.int64)

    deg = (np.bincount(dst_e, minlength=NPAD) + 1).astype(np.int32)  # + self

    core_id = (dst_e // NPC).astype(np.int32)
    tile_id = ((dst_e % NPC) // P).astype(np.int32)
    buck_id = (src_e // BUCK).astype(np.int32)

    group = (core_id.astype(np.int64) * TPC + tile_id) * NBUCK + buck_id
    counts = np.bincount(group, minlength=NC * TPC * NBUCK) \
        .reshape(NC, TPC, NBUCK)
    # NI: gathered rows per (tile,bucket) — same on all cores (SPMD), rounded
    # to the 16-index wrap granularity. REG: the 128-aligned slot region each
    # bucket occupies in the msg/S tiles (PE blocks are 128 slots).
    NI = np.maximum(_roundup(counts.max(axis=0), 16), 16)   # [TPC, NBUCK]
    REG = np.maximum(_roundup(NI, P), P)

    order = np.argsort(group, kind='stable')
    src_s = src_e[order]
    dst_s = dst_e[order]
    cnt_flat = counts  # [NC,TPC,NBUCK]
    ends = np.cumsum(cnt_flat.reshape(-1))
    starts = (ends - cnt_flat.reshape(-1)).reshape(NC, TPC, NBUCK)

    idxcols = int(NI.sum() // 16)        # int16 index columns per tile summed
    nblk = REG // P                      # [TPC, NBUCK] gather blocks
    nblk_tot = int(nblk.sum()) + TPC     # + one self block per tile
    off16 = np.zeros((TPC, NBUCK), np.int64)
    offblk = np.zeros((TPC, NBUCK), np.int64)
    acc16 = 0
    accblk = 0
    for t in range(TPC):
        for b in range(NBUCK):
            off16[t, b] = acc16
            offblk[t, b] = accblk
            acc16 += NI[t, b] // 16
            accblk += REG[t, b] // P
        accblk += 1                      # self block slot

    idx16 = np.zeros((NC, 16, idxcols), np.int16)
    dstloc = np.full((NC, P, nblk_tot), 255.0, ml_dtypes.bfloat16)
    for c in range(NC):
        for t in range(TPC):
            for b in range(NBUCK):
                n = int(cnt_flat[c, t, b])
                ni = int(NI[t, b])
                reg = int(REG[t, b])
                s0 = int(starts[c, t, b])
                sl = np.zeros(ni, np.int16)
                dl = np.full(reg, 255.0, np.float32)
                sl[:n] = (src_s[s0:s0 + n] - b * BUCK).astype(np.int16)
                dl[:n] = (dst_s[s0:s0 + n] - (c * NPC + t * P)).astype(np.float32)
                o16 = int(off16[t, b])
                idx16[c, :, o16:o16 + ni // 16] = sl.reshape(-1, 16).T
                ob = int(offblk[t, b])
                dstloc[c, :, ob:ob + reg // P] = \
                    dl.reshape(-1, P).T.astype(ml_dtypes.bfloat16)
            # self block: identity (dst_local[p] = p)
            sb = int(offblk[t, NBUCK - 1]) + int(REG[t, NBUCK - 1]) // P
            dstloc[c, :, sb] = np.arange(P, dtype=np.float32) \
                .astype(ml_dtypes.bfloat16)
    idx16_rep = np.tile(idx16, (1, 8, 1))  # [NC, 128, idxcols]

    x_pad = np.zeros((NPAD, NFEAT), np.float32)
    x_pad[:N] = np.asarray(node_features, np.float32)
    xT = np.ascontiguousarray(x_pad.T)                      # [256, NPAD]
    xT_shard = xT.reshape(NFEAT, NC, NPC).transpose(1, 0, 2).copy()

    degT = deg.reshape(NC, TPC, P).transpose(0, 2, 1).copy()  # [NC,128,TPC]

    map_pad = np.zeros(NPAD, np.int64)
    map_pad[:N] = np.asarray(node_to_comm_map)
    m16 = map_pad.reshape(NC, NPC // 16, 16).transpose(0, 2, 1).astype(np.int16)
    map16_rep = np.tile(m16, (1, 8, 1))   # [NC, 128, 784]

    meta = dict(NI=NI, REG=REG, nblk=nblk, off16=off16, offblk=offblk,
                idxcols=idxcols, nblk_tot=nblk_tot)
    return meta, idx16_rep, dstloc, xT_shard, degT, map16_rep


def _build_nc(meta, repeat=0, no_head=False, head_act=False):
    """repeat>0: wrap phases B and C in For_i(0, repeat) — timing builds only."""
    NI = meta['NI']
    REG = meta['REG']
    nblk = meta['nblk']
    off16 = meta['off16']
    offblk = meta['offblk']
    idxcols = meta['idxcols']
    nblk_tot = meta['nblk_tot']

    nc = bacc.Bacc("TRN2", target_bir_lowering=False, num_devices=NC,
                   num_swdge_queues=4)

    # I/O
    xa_dt = bf16 if XA16 else f32
    xT_d = nc.dram_tensor("xT", [NFEAT, NPC], xa_dt, kind="ExternalInput")
    idx_d = nc.dram_tensor("eidx", [128, idxcols], i16, kind="ExternalInput")
    dstloc_d = nc.dram_tensor("dstloc", [128, nblk_tot], bf16, kind="ExternalInput")
    deg_d = nc.dram_tensor("degT", [128, TPC], i32, kind="ExternalInput")
    map_d = nc.dram_tensor("map16", [128, NPC // 16], i16, kind="ExternalInput")
    comm_d = nc.dram_tensor("comm", [NCOMM, NHID], f32, kind="ExternalInput")
    W0_d = nc.dram_tensor("W0", [NFEAT, NHID], xa_dt, kind="ExternalInput")
    W1_d = nc.dram_tensor("W1", [NHID, NHID], f32, kind="ExternalInput")
    b0_d = nc.dram_tensor("b0bc", [128, NHID], bf16, kind="ExternalInput")
    b1_d = nc.dram_tensor("b1bc", [128, NHID], bf16, kind="ExternalInput")
    gwh_d = nc.dram_tensor("gwhbc", [128, NHID], f32, kind="ExternalInput")
    gwc_d = nc.dram_tensor("gwcbc", [128, NHID], f32, kind="ExternalInput")
    gb_d = nc.dram_tensor("gateb", [128, 1], f32, kind="ExternalInput")
    clsW_d = nc.dram_tensor("clsW", [NHID, NCLASS], f32, kind="ExternalInput")
    clsb_d = nc.dram_tensor("clsb", [1, NCLASS], f32, kind="ExternalInput")
    eye16_d = nc.dram_tensor("eye16", [128, 128], bf16, kind="ExternalInput")
    eye32_d = nc.dram_tensor("eye32", [128, 128], f32, kind="ExternalInput")
    iota_d = nc.dram_tensor("iota16", [128, 128], bf16, kind="ExternalInput")
    ones_d = nc.dram_tensor("ones1", [1, 128], f32, kind="ExternalInput")
    out_d = nc.dram_tensor("out", [NPC, NCLASS], f32, kind="ExternalOutput")

    # internal DRAM
    y0_shard = nc.dram_tensor("y0_shard", [NPC, NHID], bf16)
    y1_shard = nc.dram_tensor("y1_shard", [NPC, NHID], bf16)
    y0_full = nc.dram_tensor("y0_full", [NPAD, NHID], bf16, addr_space="Shared")
    y1_full = nc.dram_tensor("y1_full", [NPAD, NHID], bf16, addr_space="Shared")

    RG = [list(range(NC))]
    slots_t = [int(REG[t].sum()) + P for t in range(TPC)]   # + self region
    max_slots = max(slots_t)

    with tile.TileContext(nc) as tc:
        with tc.tile_pool(name="const", bufs=1) as cp, \
             tc.tile_pool(name="work", bufs=3) as wp, \
             tc.tile_pool(name="psum", bufs=2, space="PSUM") as pp:

            # ---- constants into SBUF
            def cload(dram, shape, dtype, name):
                t_ = cp.tile(shape, dtype, name=name)
                nc.sync.dma_start(out=t_[:], in_=dram[:, :])
                return t_

            W0t = cp.tile([128, 2 * NHID], xa_dt, name="W0t")
            nc.sync.dma_start(out=W0t[:, :NHID], in_=W0_d[0:128, :])
            nc.sync.dma_start(out=W0t[:, NHID:], in_=W0_d[128:256, :])
            W1t = cload(W1_d, [128, NHID], f32, "W1t")
            b0t = cload(b0_d, [128, NHID], bf16, "b0t")
            b1t = cload(b1_d, [128, NHID], bf16, "b1t")
            gwht = cload(gwh_d, [128, NHID], f32, "gwht")
            gwct = cload(gwc_d, [128, NHID], f32, "gwct")
            gbt = cload(gb_d, [128, 1], f32, "gbt")
            clsWt = cload(clsW_d, [NHID, NCLASS], f32, "clsWt")
            clsbt = cload(clsb_d, [1, NCLASS], f32, "clsbt")
            eye16 = cload(eye16_d, [128, 128], bf16, "eye16")
            eye32 = cload(eye32_d, [128, 128], f32, "eye32")
            iota16 = cload(iota_d, [128, 128], bf16, "iota16")
            ones1 = cload(ones_d, [1, 128], f32, "ones1")
            dstloc_all = cload(dstloc_d, [128, nblk_tot], bf16, "dstloc_all")
            map16 = cload(map_d, [128, NPC // 16], i16, "map16")
            degt_i = cload(deg_d, [128, TPC], i32, "degt_i")

            deg_f = cp.tile([128, TPC], f32, name="deg_f")
            nc.vector.tensor_copy(out=deg_f[:], in_=degt_i[:])
            deg_r = cp.tile([128, TPC], f32, name="deg_r")
            nc.vector.reciprocal(out=deg_r[:], in_=deg_f[:])
            dinv = cp.tile([128, TPC], f32, name="dinv")
            nc.scalar.activation(out=dinv[:], in_=deg_r[:], func=AF.Sqrt)
            sqd = cp.tile([128, TPC], f32, name="sqd")
            nc.scalar.activation(out=sqd[:], in_=deg_f[:], func=AF.Sqrt)

            # ---- hc gather (comm_features[node_to_comm_map]) -> resident
            hc_all = cp.tile([128, NPC], f32, name="hc_all")
            goff = 0
            qn = 0
            while goff < NPC:
                gn = min(2048, NPC - goff)
                nc.gpsimd.dma_gather(
                    out_ap=hc_all[:, goff:goff + gn]
                        .rearrange("p (k d) -> p k d", d=NHID),
                    in_ap=comm_d[:, :],
                    idxs_ap=map16[:, goff // 16:(goff + gn) // 16],
                    num_idxs=gn, num_idxs_reg=gn, elem_size=NHID,
                    single_packet=False, queue_num=qn % 4,
                )
                qn += 1
                goff += gn

            # ---- Phase A: y0 = dinv * (x @ W0)  (node-major, bf16 out)
            # xT loaded in 8-tile chunks (two 512 KB DMAs each) for bandwidth.
            CH = 8
            chunks = [(g * CH, min(CH, TPC - g * CH))
                      for g in range((TPC + CH - 1) // CH)]

            def phase_a():
              for (t0, ct) in chunks:
                xta = wp.tile([128, CH * P], xa_dt, tag="xta", bufs=2)
                xtb = wp.tile([128, CH * P], xa_dt, tag="xtb", bufs=2)
                nc.sync.dma_start(out=xta[:, :ct * P],
                                  in_=xT_d[0:128, t0 * P:(t0 + ct) * P])
                nc.sync.dma_start(out=xtb[:, :ct * P],
                                  in_=xT_d[128:256, t0 * P:(t0 + ct) * P])
                for j in range(ct):
                    t = t0 + j
                    psz = pp.tile([128, NHID], f32, tag="psz")
                    nc.tensor.matmul(psz[:], lhsT=xta[:, j * P:(j + 1) * P],
                                     rhs=W0t[:, :NHID], start=True, stop=False)
                    nc.tensor.matmul(psz[:], lhsT=xtb[:, j * P:(j + 1) * P],
                                     rhs=W0t[:, NHID:], start=False, stop=True)
                    y0sb = wp.tile([128, NHID], bf16, tag="y0sb")
                    nc.scalar.activation(out=y0sb[:], in_=psz[:], func=AF.Copy,
                                         scale=dinv[:, t:t + 1])
                    nc.sync.dma_start(out=y0_shard[t * P:(t + 1) * P, :],
                                      in_=y0sb[:])

            if repeat:
                with tc.For_i(0, repeat, 1):
                    phase_a()
            else:
                phase_a()

            nc.gpsimd.collective_compute(
                "AllGather", OP.bypass, replica_groups=RG,
                ins=[y0_shard[:, :]], outs=[y0_full[:, :]])

            # one-time zero-fill of the msg slots: gathers with num_idxs not a
            # multiple of 128 leave tail slots unwritten; they multiply S=0 in
            # the matmul, which is only safe if the stale bits are finite.
            for _i in range(4):
                mz = wp.tile([128, max_slots], bf16, tag="msg", bufs=4,
                             name=f"msgz{_i}")
                nc.vector.memset(mz[:], 0)

            # ---- shared aggregation tile body
            def agg_tile(t, y_full, y_shard_cur, bbc_t, psum_tag):
                icols = int(NI[t].sum() // 16)
                i0 = int(off16[t, 0])
                idx_t = wp.tile([128, icols], i16, tag="idx", bufs=6)
                nc.sync.dma_start(out=idx_t[:], in_=idx_d[:, i0:i0 + icols])
                st = slots_t[t]
                nb = st // P
                msg = wp.tile([128, st], bf16, tag="msg", bufs=4)
                eoff = 0
                ioff = 0
                for b in range(NBUCK):
                    nib = int(NI[t, b])
                    reg = int(REG[t, b])
                    nc.gpsimd.dma_gather(
                        out_ap=msg[:, eoff:eoff + reg]
                            .rearrange("p (k d) -> p k d", d=NHID),
                        in_ap=y_full[b * BUCK:(b + 1) * BUCK, :],
                        idxs_ap=idx_t[:, ioff:ioff + nib // 16],
                        num_idxs=nib, num_idxs_reg=nib, elem_size=NHID,
                        single_packet=False, queue_num=b,
                    )
                    eoff += reg
                    ioff += nib // 16
                # self rows: contiguous from this core's own y_shard
                nc.sync.dma_start(out=msg[:, eoff:eoff + P],
                                  in_=y_shard_cur[t * P:(t + 1) * P, :])
                # S build: S[p, kb, j] = (iota[p, j] == dstloc[p, kb])
                S = wp.tile([128, st], bf16, tag="S", bufs=3)
                ob = int(offblk[t, 0])
                ibase = iota16[:]
                dbase = dstloc_all[:, ob:ob + nb]
                iota_b = bass.AP(ibase.tensor, ibase.offset,
                                 [list(ibase.ap[0]), [0, nb], list(ibase.ap[1])])
                dst_b = bass.AP(dbase.tensor, dbase.offset,
                                [list(dbase.ap[0]), list(dbase.ap[1]), [0, 128]])
                nc.vector.tensor_tensor(
                    out=S[:].rearrange("p (k d) -> p k d", d=128),
                    in0=iota_b, in1=dst_b, op=OP.is_equal)
                Sb = wp.tile([128, 128], bf16, tag="Sb", bufs=2)
                nc.vector.tensor_scalar(out=Sb[:], in0=eye16[:],
                                        scalar1=sqd[:, t:t + 1], scalar2=None,
                                        op0=OP.mult)
                ph = pp.tile([128, NHID], f32, tag=psum_tag)
                for kb in range(nb):
                    nc.tensor.matmul(ph[:], lhsT=S[:, kb * P:(kb + 1) * P],
                                     rhs=msg[:, kb * P:(kb + 1) * P],
                                     start=(kb == 0), stop=False)
                nc.tensor.matmul(ph[:], lhsT=Sb[:], rhs=bbc_t[:],
                                 start=False, stop=True)
                h = wp.tile([128, NHID], f32, tag="h", bufs=4)
                nc.scalar.activation(out=h[:], in_=ph[:], func=AF.Relu,
                                     scale=dinv[:, t:t + 1])
                return h

            # ---- Phase B: layer 1 aggregation + z1 + y1
            def phase_b():
                for t in range(TPC):
                    h1 = agg_tile(t, y0_full, y0_shard, b0t, "ph1")
                    ptr = pp.tile([128, 128], f32, tag="ptr")
                    nc.tensor.transpose(ptr[:], h1[:], eye32[:])
                    h1T = wp.tile([128, 128], f32, tag="h1T", bufs=3)
                    if head_act:
                        nc.scalar.activation(out=h1T[:], in_=ptr[:], func=AF.Copy)
                    else:
                        nc.vector.tensor_copy(out=h1T[:], in_=ptr[:])
                    psz1 = pp.tile([128, NHID], f32, tag="psz")
                    nc.tensor.matmul(psz1[:], lhsT=h1T[:], rhs=W1t[:],
                                     start=True, stop=True)
                    y1sb = wp.tile([128, NHID], bf16, tag="y0sb")
                    nc.scalar.activation(out=y1sb[:], in_=psz1[:], func=AF.Copy,
                                         scale=dinv[:, t:t + 1])
                    nc.sync.dma_start(out=y1_shard[t * P:(t + 1) * P, :],
                                      in_=y1sb[:])

            if repeat:
                with tc.For_i(0, repeat, 1):
                    phase_b()
            else:
                phase_b()

            nc.gpsimd.collective_compute(
                "AllGather", OP.bypass, replica_groups=RG,
                ins=[y1_shard[:, :]], outs=[y1_full[:, :]])

            # ---- Phase C: layer 2 aggregation + gate + classifier + lsm
            def phase_c():
              for t in range(TPC):
                h2 = agg_tile(t, y1_full, y1_shard, b1t, "ph1")
                if no_head:
                    nc.sync.dma_start(out=out_d[t * P:(t + 1) * P, :],
                                      in_=h2[:, :NCLASS])
                    continue
                hc_t = hc_all[:, t * P:(t + 1) * P]
                tmp = wp.tile([128, NHID], f32, tag="tmp", bufs=2)
                u1 = wp.tile([128, 1], f32, tag="u1", bufs=2)
                nc.vector.tensor_tensor(out=tmp[:], in0=h2[:], in1=gwht[:],
                                        op=OP.mult)
                nc.vector.reduce_sum(out=u1[:], in_=tmp[:],
                                     axis=mybir.AxisListType.X)
                tmp2 = wp.tile([128, NHID], f32, tag="tmp2", bufs=2)
                u2 = wp.tile([128, 1], f32, tag="u2", bufs=2)
                nc.vector.tensor_tensor(out=tmp2[:], in0=hc_t, in1=gwct[:],
                                        op=OP.mult)
                nc.vector.reduce_sum(out=u2[:], in_=tmp2[:],
                                     axis=mybir.AxisListType.X)
                u = wp.tile([128, 1], f32, tag="u", bufs=2)
                nc.vector.tensor_tensor(out=u[:], in0=u1[:], in1=u2[:], op=OP.add)
                w = wp.tile([128, 1], f32, tag="w", bufs=2)
                nc.scalar.activation(out=w[:], in_=u[:], func=AF.Sigmoid,
                                     bias=gbt[:, 0:1])
                d = wp.tile([128, NHID], f32, tag="d", bufs=2)
                nc.vector.tensor_tensor(out=d[:], in0=h2[:], in1=hc_t, op=OP.subtract)
                dw = wp.tile([128, NHID], f32, tag="dw", bufs=2)
                if head_act:
                    nc.scalar.activation(out=dw[:], in_=d[:], func=AF.Copy,
                                         scale=w[:, 0:1])
                else:
                    nc.vector.tensor_scalar(out=dw[:], in0=d[:], scalar1=w[:, 0:1],
                                            scalar2=None, op0=OP.mult)
                hf = wp.tile([128, NHID], f32, tag="hf", bufs=2)
                nc.vector.tensor_tensor(out=hf[:], in0=dw[:], in1=hc_t, op=OP.add)
                ptr2 = pp.tile([128, 128], f32, tag="ptr")
                nc.tensor.transpose(ptr2[:], hf[:], eye32[:])
                hfT = wp.tile([128, 128], f32, tag="h1T", bufs=3)
                nc.vector.tensor_copy(out=hfT[:], in_=ptr2[:])
                pc = pp.tile([128, NCLASS], f32, tag="pc")
                nc.tensor.matmul(pc[:], lhsT=hfT[:], rhs=clsWt[:],
                                 start=True, stop=False)
                nc.tensor.matmul(pc[:], lhsT=ones1[:], rhs=clsbt[:],
                                 start=False, stop=True)
                mx = wp.tile([128, 1], f32, tag="mx", bufs=2)
                nc.vector.reduce_max(out=mx[:], in_=pc[:],
                                     axis=mybir.AxisListType.X)
                if head_act:
                    negmx = wp.tile([128, 1], f32, tag="negmx", bufs=3)
                    nc.vector.tensor_scalar(out=negmx[:], in0=mx[:], scalar1=-1.0,
                                            scalar2=None, op0=OP.mult)
                    ex = wp.tile([128, NCLASS], f32, tag="ex", bufs=3)
                    nc.scalar.activation(out=ex[:], in_=pc[:], func=AF.Exp,
                                         bias=negmx[:, 0:1])
                    ssum = wp.tile([128, 1], f32, tag="ssum", bufs=3)
                    nc.vector.reduce_sum(out=ssum[:], in_=ex[:],
                                         axis=mybir.AxisListType.X)
                    rcp = wp.tile([128, 1], f32, tag="rcp", bufs=3)
                    nc.vector.reciprocal(out=rcp[:], in_=ssum[:])
                    nls = wp.tile([128, 1], f32, tag="nls", bufs=3)
                    nc.scalar.activation(out=nls[:], in_=rcp[:], func=AF.Ln)
                    bias2 = wp.tile([128, 1], f32, tag="bias2", bufs=3)
                    nc.vector.tensor_tensor(out=bias2[:], in0=negmx[:],
                                            in1=nls[:], op=OP.add)
                    outt = wp.tile([128, NCLASS], f32, tag="outt", bufs=3)
                    nc.scalar.activation(out=outt[:], in_=pc[:],
                                         func=AF.Identity, bias=bias2[:, 0:1])
                else:
                    xm = wp.tile([128, NCLASS], f32, tag="xm", bufs=2)
                    nc.vector.tensor_scalar(out=xm[:], in0=pc[:],
                                            scalar1=mx[:, 0:1],
                                            scalar2=None, op0=OP.subtract)
                    ex = wp.tile([128, NCLASS], f32, tag="ex", bufs=2)
                    nc.scalar.activation(out=ex[:], in_=xm[:], func=AF.Exp)
                    ssum = wp.tile([128, 1], f32, tag="ssum", bufs=2)
                    nc.vector.reduce_sum(out=ssum[:], in_=ex[:],
                                         axis=mybir.AxisListType.X)
                    lns = wp.tile([128, 1], f32, tag="lns", bufs=2)
                    nc.scalar.activation(out=lns[:], in_=ssum[:], func=AF.Ln)
                    outt = wp.tile([128, NCLASS], f32, tag="outt", bufs=3)
                    nc.vector.tensor_scalar(out=outt[:], in0=xm[:],
                                            scalar1=lns[:, 0:1], scalar2=None,
                                            op0=OP.subtract)
                nc.sync.dma_start(out=out_d[t * P:(t + 1) * P, :], in_=outt[:])

            if repeat:
                with tc.For_i(0, repeat, 1):
                    phase_c()
            else:
                phase_c()

    nc.compile()
    return nc


def _make_in_maps(inputs, meta, idx16_rep, dstloc, xT_shard, degT, map16_rep):
    gate_W = np.asarray(inputs["gate_W"], np.float32)
    shared = {
        "comm": np.asarray(inputs["comm_features"], np.float32),
        "W0": (np.asarray(inputs["W0"], np.float32).astype(ml_dtypes.bfloat16)
               if XA16 else np.asarray(inputs["W0"], np.float32)),
        "W1": np.asarray(inputs["W1"], np.float32),
        "b0bc": np.tile(np.asarray(inputs["b0"], np.float32), (128, 1)).astype(ml_dtypes.bfloat16),
        "b1bc": np.tile(np.asarray(inputs["b1"], np.float32), (128, 1)).astype(ml_dtypes.bfloat16),
        "gwhbc": np.tile(gate_W[:NHID, 0], (128, 1)),
        "gwcbc": np.tile(gate_W[NHID:, 0], (128, 1)),
        "gateb": np.full((128, 1), float(np.asarray(inputs["gate_b"]).reshape(-1)[0]), np.float32),
        "clsW": np.asarray(inputs["cls_W"], np.float32),
        "clsb": np.asarray(inputs["cls_b"], np.float32).reshape(1, NCLASS),
        "eye16": np.eye(128, dtype=np.float32).astype(ml_dtypes.bfloat16),
        "eye32": np.eye(128, dtype=np.float32),
        "iota16": np.tile(np.arange(128, dtype=np.float32), (128, 1)).astype(ml_dtypes.bfloat16),
        "ones1": np.ones((1, 128), np.float32),
    }
    in_maps = []
    for c in range(NC):
        m = dict(shared)
        m["xT"] = (xT_shard[c].astype(ml_dtypes.bfloat16)
                   if XA16 else xT_shard[c])
        m["eidx"] = idx16_rep[c]
        m["dstloc"] = np.asarray(dstloc[c])
        m["degT"] = degT[c]
        m["map16"] = map16_rep[c]
        in_maps.append(m)
    return in_maps


def kernel(node_features, node_adj, comm_features, comm_adj, node_to_comm_map,
           W0, b0, W1, b1, gate_W, gate_b, cls_W, cls_b):
    t0 = time.perf_counter()
    meta, idx16_rep, dstloc, xT_shard, degT, map16_rep = _host_prep(
        node_features, node_adj, node_to_comm_map)
    t1 = time.perf_counter()

    key = "nc"
    if key not in _cache:
        _cache[key] = _build_nc(meta)
    nc = _cache[key]
    t2 = time.perf_counter()

    inputs = dict(comm_features=comm_features, W0=W0, W1=W1, b0=b0, b1=b1,
                  gate_W=gate_W, gate_b=gate_b, cls_W=cls_W, cls_b=cls_b)
    in_maps = _make_in_maps(inputs, meta, idx16_rep, dstloc, xT_shard, degT,
                            map16_rep)

    res = run_bass_kernel_spmd(nc, in_maps, core_ids=list(range(NC)))
    t3 = time.perf_counter()

    out = np.concatenate([res.results[c]["out"] for c in range(NC)], axis=0)
    print(f"[kernel] host_prep={t1-t0:.2f}s build+compile={t2-t1:.2f}s "
          f"run={t3-t2:.2f}s", file=sys.stderr)
    return out[:N]
